# revision 39
# baseline (speedup 1.0000x reference)
"""Cosformer attention (causal linear attention with cos reweighting) on 8
Trainium2 NeuronCores.

Sharding: n = bsz*heads = 16 sequences -> 2 per core. Core c handles batch-half
i = c//4 and head-pair p = c%4 (heads 2p, 2p+1). Fully data/head parallel; the
only cross-core interaction is the host-side sum of output-projection partials.

v2 design notes:
  - Intra-chunk scores use RAW 64-dim q,k per head with the cos reweighting
    folded into the mask table: q_.k_ = q.k * cos(idx_t - idx_s), so
    maskcos[s,t] = triu * cos(pi/2 (t-s)/L). This kills the k feature-major
    expansion entirely and halves the score contraction depth.
  - sin/cos tables (scb/scbf for the q expansion, per-chunk sincol columns
    for the token-major k expansion) are host-precomputed and DMA'd: no
    iota, no ACT Sin, no Sin table load.
  - v is projected TOKEN-major (lhsT = x chunk, rhs = Wv) straight into the
    per-chunk (tokens, 65) layout, killing all 8 v transposes.
  - The serial S-state chain is replaced by 16 independent P_c = K_c^T V_c
    matmuls (4 chunks packed per bf16 PSUM bank) + a depth-3 prefix add tree
    on DVE, so the 8 attention chunk blocks are fully independent.
  - 5 input DMAs / 4 output DMAs total, big contiguous per-partition rows.
  - o-projection emits bf16 PSUM (single-group) for cheap packed evacuation,
    alternating ACT/DVE; outputs ship in (128,1024) pairs on the sync ring.
"""

import os
import sys

import numpy as np

for _p in ("/opt/trn_rl_repo", "/root/.axon_site/_ro/trn_rl_repo"):
    if os.path.isdir(_p) and _p not in sys.path:
        sys.path.insert(0, _p)

N_HEAD = 8
E = 512
L = 1024
BSZ = 2
D = 64
P = 128
NCHUNK = L // P
N_CORES = 8
TH = 512  # token-half width for q/k projections

# c1a: [bias 4 | wq 512 | wk 512] — only what the first projections need
_C1_BIAS = 0
_C1_WQ = 4
_C1_WK = 516
_C1A_COLS = 1028
# c1b: [wv 512 | wo 512 | ident 128 | maskcos 128 | sctab 16]
_C1B_WV = 0
_C1B_WO = 512
_C1B_IDENT = 1024
_C1B_MASK = 1152
_C1B_SCTAB = 1280
_C1B_COLS = 1296
# c3: [scb 1024 | scbf 1024]
_C3_COLS = 2048

_CACHE = {}


def _build_bass():
    import concourse.bass as bass
    import concourse.tile as tile
    from concourse import bacc, mybir
    from concourse.bass import _add_dep_helper
    from contextlib import ExitStack

    f32 = mybir.dt.float32
    bf16 = mybir.dt.bfloat16
    AF = mybir.ActivationFunctionType

    nc = bacc.Bacc("TRN2", target_bir_lowering=False, debug=False)

    xt_d = nc.dram_tensor("xt", [P, 4096], bf16, kind="ExternalInput")
    c1a_d = nc.dram_tensor("c1a", [P, _C1A_COLS], bf16, kind="ExternalInput")
    c1b_d = nc.dram_tensor("c1b", [P, _C1B_COLS], bf16, kind="ExternalInput")
    c3_d = nc.dram_tensor("c3", [P, _C3_COLS], bf16, kind="ExternalInput")
    out_d = nc.dram_tensor("out", [P, 4096], bf16, kind="ExternalOutput")

    with tile.TileContext(nc) as tc:
        with ExitStack() as ctx:
            ep = ctx.enter_context
            cpool = ep(tc.tile_pool(name="const", bufs=1))
            ktp = ep(tc.tile_pool(name="ktok", bufs=6))
            bp = ep(tc.tile_pool(name="bsb", bufs=3))
            pp = ep(tc.tile_pool(name="psb", bufs=4))
            spool = ep(tc.tile_pool(name="Ssb", bufs=10))
            ap_pool = ep(tc.tile_pool(name="apair", bufs=3))
            atp = ep(tc.tile_pool(name="attnT", bufs=3))
            outp = ep(tc.tile_pool(name="outsb", bufs=2))
            rp = ep(tc.tile_pool(name="rcol", bufs=6))
            big_ps = ep(tc.tile_pool(name="bigps", bufs=2, space="PSUM"))
            vs_ps = ep(tc.tile_pool(name="vsps", bufs=2, space="PSUM"))
            qkv_ps = ep(tc.tile_pool(name="qkvps", bufs=2, space="PSUM"))
            p_ps = ep(tc.tile_pool(name="pps", bufs=2, space="PSUM"))

            # ---- loads: scalar ring carries weights/tables, sync ring x ----
            c1a_t = cpool.tile([P, _C1A_COLS], bf16, name="c1a_t")
            c1b_t = cpool.tile([P, _C1B_COLS], bf16, name="c1b_t")
            c3_t = cpool.tile([P, _C3_COLS], bf16, name="c3_t")
            xt_sb = cpool.tile([P, 4096], bf16, name="xt_sb")
            # ONE strictly-ordered input queue (sync ring), critical bytes
            # first: the first projections gate on c1a+xt_e01 = 520KB only.
            # Outputs go on the scalar ring (no FIFO entanglement).
            dma0 = nc.sync.dma_start(c1a_t[:], c1a_d[:, :])
            nc.sync.dma_start(xt_sb[:, 0:1024], xt_d[:, 0:1024])
            nc.sync.dma_start(xt_sb[:, 1024:2048], xt_d[:, 1024:2048])
            nc.sync.dma_start(xt_sb[:, 2048:4096], xt_d[:, 2048:4096])
            nc.sync.dma_start(c1b_t[:], c1b_d[:, :])
            nc.sync.dma_start(c3_t[:], c3_d[:, :])

            wq = [c1a_t[:, _C1_WQ + e * P : _C1_WQ + (e + 1) * P] for e in range(4)]
            wk = [c1a_t[:, _C1_WK + e * P : _C1_WK + (e + 1) * P] for e in range(4)]
            wv = [c1b_t[:, _C1B_WV + e * P : _C1B_WV + (e + 1) * P] for e in range(4)]
            wo_t = c1b_t[:, _C1B_WO : _C1B_WO + 512]
            ident_t = c1b_t[:, _C1B_IDENT : _C1B_IDENT + P]
            maskcos_t = c1b_t[:, _C1B_MASK : _C1B_MASK + P]
            sctab = c1b_t[:, _C1B_SCTAB : _C1B_SCTAB + 16]  # (128,16) bf16 sin/cos
            bq_col = c1a_t[:, 0:2].bitcast(f32)
            bk_col = c1a_t[:, 2:4].bitcast(f32)
            scb = c3_t[:, 0:1024]
            scbf = c3_t[:, 1024:2048]

            def xslice(th, e):
                return xt_sb[:, 2048 * th + 512 * e : 2048 * th + 512 * (e + 1)]

            # ---- persistent activations ----
            # qz_a: head-a raw q on partitions 0:64, zeros elsewhere; qz_b:
            # head-b on 64:128, zeros elsewhere — so per-head scores can use
            # full-K=128 matmuls (legal same-bank packing, shared LDWEIGHTS).
            qz = {h: cpool.tile([P, L], bf16, name=f"qz_{h}") for h in "ab"}
            kp_sb = cpool.tile([P, L], bf16, name="kp_sb")
            q_seq = {h: cpool.tile([P, L], bf16, name=f"q_{h}") for h in "ab"}
            vt_all = cpool.tile([P, NCHUNK * 2 * (D + 1)], bf16, name="vt_all")
            # only the ones-columns need the memset (v cols get overwritten);
            # DVE, deferred past the first DMA so the exec-time window and the
            # early DMA stream stay clean.
            mset = nc.vector.memset(
                vt_all[:].rearrange("p (c e) -> p c e", c=2 * NCHUNK, e=65)[:, :, 64:65],
                1.0,
            )
            _add_dep_helper(mset.ins, dma0.ins, sync=True, reason="late memset")
            mza = nc.vector.memset(qz["a"][D:P, :], 0.0)
            _add_dep_helper(mza.ins, dma0.ins, sync=True, reason="late memset")
            mzb = nc.vector.memset(qz["b"][0:D, :], 0.0)
            _add_dep_helper(mzb.ins, dma0.ins, sync=True, reason="late memset")

            def proj_qk(th, what):
                sl = slice(th * TH, (th + 1) * TH)
                ps = big_ps.tile([P, TH], f32, tag="big", name=f"{what}_ps{th}")
                w = wq if what == "q" else wk
                for e in range(4):
                    nc.tensor.matmul(
                        ps[:], w[e], xslice(th, e), start=(e == 0), stop=(e == 3)
                    )
                if what == "q":
                    nc.scalar.activation(
                        qz["a"][0:D, sl], ps[0:D, :], AF.Relu, bias=bq_col[0:D, 0:1]
                    )
                    nc.scalar.activation(
                        qz["b"][D:P, sl], ps[D:P, :], AF.Relu, bias=bq_col[D:P, 0:1]
                    )
                else:
                    nc.scalar.activation(
                        kp_sb[:, sl], ps[:], AF.Relu, bias=bk_col[:, 0:1]
                    )

            def expand_q(th):
                sl = slice(th * TH, (th + 1) * TH)
                nc.vector.tensor_mul(q_seq["a"][0:D, sl], qz["a"][0:D, sl], scb[0:D, sl])
                nc.vector.tensor_mul(q_seq["a"][D:P, sl], qz["a"][0:D, sl], scbf[0:D, sl])
                nc.vector.tensor_mul(q_seq["b"][0:D, sl], qz["b"][D:P, sl], scbf[D:P, sl])
                nc.vector.tensor_mul(q_seq["b"][D:P, sl], qz["b"][D:P, sl], scb[D:P, sl])

            def proj_v_pair(c0):
                """Project chunks c0, c0+1 token-major into one PSUM bank."""
                v_ps = vs_ps.tile([P, 2 * P], f32, tag="vs", name=f"vps{c0}")
                for j in range(2):
                    th, lc = divmod(c0 + j, 4)
                    for e in range(4):
                        lhsT = xt_sb[
                            :,
                            2048 * th + 512 * e + P * lc : 2048 * th + 512 * e + P * (lc + 1),
                        ]
                        nc.tensor.matmul(
                            v_ps[:, j * P : (j + 1) * P], lhsT, wv[e],
                            start=(j == 0 and e == 0),
                            stop=(j == 1 and e == 3),
                            skip_group_check=True,
                        )
                nc.scalar.copy(
                    vt_all[:, c0 * 130 : (c0 + 2) * 130]
                    .rearrange("p (h e) -> p h e", h=4, e=65)[:, :, 0:64],
                    v_ps[:].rearrange("p (h e) -> p h e", h=4, e=64),
                )

            ktoks = {}

            def kt_chunk(c):
                cs = slice(c * P, (c + 1) * P)
                ktr = vs_ps.tile([P, P], bf16, tag="vs", name=f"ktr{c}")
                nc.tensor.matmul(ktr[:], kp_sb[:, cs], ident_t, is_transpose=True)
                ktok = ktp.tile([P, 2 * P], bf16, tag="ktok", name=f"ktok{c}")
                # ktok = [a*sin | a*cos | b*sin | b*cos] in ONE op: stride-0
                # read of ktr per (sin,cos), stride-0 bcast of the per-token
                # sin/cos column pair along d.
                nc.vector.tensor_mul(
                    ktok[:].rearrange("p (h s d) -> p h s d", h=2, s=2, d=64),
                    ktr[:].rearrange("p (h d) -> p h d", h=2, d=64)
                    .unsqueeze(2)
                    .broadcast_to([P, 2, 2, 64]),
                    sctab[:, 2 * c : 2 * c + 2]
                    .unsqueeze(1)
                    .unsqueeze(3)
                    .broadcast_to([P, 2, 2, 64]),
                )
                ktoks[c] = ktok

            def p_mms(cs2, name):
                """P_c = K_c_^T V_c_aug for 2 chunks into one f32 bank."""
                ps = p_ps.tile([P, 2 * 130], f32, tag="pp", name=name)
                n = len(cs2) * 2
                i = 0
                for j, c in enumerate(cs2):
                    for h in range(2):
                        # one start (bank has_written clear), one stop;
                        # skip_group_check everywhere so the sim's
                        # per-region group tracker stays out of the way.
                        nc.tensor.matmul(
                            ps[:, j * 130 + h * 65 : j * 130 + (h + 1) * 65],
                            ktoks[c][:, h * P : (h + 1) * P],
                            vt_all[:, c * 130 + h * 65 : c * 130 + (h + 1) * 65],
                            start=(i == 0),
                            stop=(i == n - 1),
                            skip_group_check=True,
                        )
                        i += 1
                return ps

            def sadd(name, a, b, eng=None):
                t = spool.tile([P, 130], bf16, tag="S", name=name)
                (eng or nc.vector).tensor_add(t[:], a, b)
                return t

            o_sbs = [
                outp.tile([P, 2 * E], bf16, tag="osb", name=f"osb{i}") for i in range(2)
            ]

            def attn_chunk(c, S_prev):
                cs = slice(c * P, (c + 1) * P)
                sc = vs_ps.tile([P, 2 * P], f32, tag="vs", name=f"sc{c}")
                nc.tensor.matmul(
                    sc[:, 0:P], kp_sb[:, cs], qz["a"][:, cs],
                    start=True, stop=False, skip_group_check=True,
                )
                nc.tensor.matmul(
                    sc[:, P : 2 * P], kp_sb[:, cs], qz["b"][:, cs],
                    start=False, stop=True, skip_group_check=True,
                )
                b_sb = bp.tile([P, 2 * P], bf16, tag="bsb", name=f"bsb{c}")
                nc.vector.tensor_mul(
                    b_sb[:].rearrange("p (h t) -> p h t", h=2, t=P),
                    sc[:].rearrange("p (h t) -> p h t", h=2, t=P),
                    maskcos_t.unsqueeze(1).broadcast_to([P, 2, P]),
                )
                qkv = qkv_ps.tile([P, 2 * (D + 1)], f32, tag="qkv", name=f"qkv{c}")
                nc.tensor.matmul(
                    qkv[:, 0:65], b_sb[:, 0:P], vt_all[:, c * 130 : c * 130 + 65],
                    start=True, stop=False, skip_group_check=True,
                )
                nc.tensor.matmul(
                    qkv[:, 65:130], b_sb[:, P : 2 * P],
                    vt_all[:, c * 130 + 65 : (c + 1) * 130],
                    start=False, stop=(c == 0), skip_group_check=True,
                )
                if c > 0:
                    nc.tensor.matmul(
                        qkv[:, 0:65], q_seq["a"][:, cs], S_prev[:, 0:65],
                        start=False, stop=False, skip_group_check=True,
                    )
                    nc.tensor.matmul(
                        qkv[:, 65:130], q_seq["b"][:, cs], S_prev[:, 65:130],
                        start=False, stop=True, skip_group_check=True,
                    )
                r_col = rp.tile([P, 4], f32, tag="r", name=f"r{c}")
                nc.vector.reciprocal(r_col[:, 2:4], qkv[:, 64:130:65])
                attn_pair = ap_pool.tile([P, P], bf16, tag="ap", name=f"ap{c}")
                nc.vector.tensor_mul(
                    attn_pair[:].rearrange("p (h f) -> p h f", h=2, f=64),
                    qkv[:].rearrange("p (h f) -> p h f", h=2, f=65)[:, :, 0:64],
                    r_col[:, 2:4].unsqueeze(2).broadcast_to([P, 2, 64]),
                )
                at_ps = p_ps.tile([P, P], bf16, tag="pp", name=f"atps{c}")
                nc.tensor.matmul(at_ps[:], attn_pair[:], ident_t, is_transpose=True)
                at_sb = atp.tile([P, P], bf16, tag="at", name=f"at{c}")
                nc.vector.tensor_copy(at_sb[:], at_ps[:])
                o_ps = big_ps.tile([P, E], f32, tag="big", name=f"ops{c}")
                nc.tensor.matmul(o_ps[:], at_sb[:], wo_t, start=True, stop=True)
                o_sb = o_sbs[(c // 2) % 2]
                osl = o_sb[:, (c % 2) * E : (c % 2 + 1) * E]
                nc.scalar.copy(osl, o_ps[:])
                if c >= 6:
                    # tail chunks ship individually so the last transfer is
                    # small and the drain phase starts early
                    nc.scalar.dma_start(out_d[:, c * E : (c + 1) * E], osl)
                elif c % 2 == 1:
                    nc.scalar.dma_start(out_d[:, (c - 1) * E : (c + 1) * E], o_sb[:])

            # ---- program order (== scheduler priority) ----
            proj_qk(0, "q")
            expand_q(0)
            proj_qk(0, "k")
            proj_v_pair(0)
            proj_v_pair(2)
            # th0-dependent chunk setup first: fills the PE while xt_th1/c1b
            # are still landing
            for c in range(4):
                kt_chunk(c)
            ps01 = p_mms((0, 1), "P01ps")
            p01 = pp.tile([P, 260], bf16, tag="P", name="P01")
            nc.scalar.copy(p01[:], ps01[:])
            attn_chunk(0, None)
            attn_chunk(1, p01[:, 0:130])
            ps23 = p_mms((2, 3), "P23ps")
            p23 = pp.tile([P, 260], bf16, tag="P", name="P23")
            nc.scalar.copy(p23[:], ps23[:])
            proj_qk(1, "q")
            expand_q(1)
            proj_qk(1, "k")
            proj_v_pair(4)
            proj_v_pair(6)
            for c in range(4, 8):
                kt_chunk(c)
            ps45 = p_mms((4, 5), "P45ps")
            p45 = pp.tile([P, 260], bf16, tag="P", name="P45")
            nc.scalar.copy(p45[:], ps45[:])
            ps67 = p_mms((6, 7), "P67ps")
            p67 = pp.tile([P, 260], bf16, tag="P", name="P67")
            nc.scalar.copy(p67[:], ps67[:])

            # prefix tree: S_c = sum_{j<=c} P_j  (bf16 adds, depth <= 3)
            s1 = sadd("S1", p01[:, 0:130], p01[:, 130:260])
            a23 = sadd("A23", p23[:, 0:130], p23[:, 130:260], nc.gpsimd)
            s2 = sadd("S2", s1[:], p23[:, 0:130])
            s3 = sadd("S3", s1[:], a23[:])
            a45 = sadd("A45", p45[:, 0:130], p45[:, 130:260], nc.gpsimd)
            a456 = sadd("A456", a45[:], p67[:, 0:130], nc.gpsimd)
            s4 = sadd("S4", s3[:], p45[:, 0:130])
            s5 = sadd("S5", s3[:], a45[:])
            s6 = sadd("S6", s3[:], a456[:])
            S_list = [p01[:, 0:130], s1[:], s2[:], s3[:], s4[:], s5[:], s6[:]]

            for c in range(2, NCHUNK):
                attn_chunk(c, S_list[c - 1])

    nc.compile()
    return nc


def _get_nc():
    if "nc" not in _CACHE:
        _CACHE["nc"] = _build_bass()
    return _CACHE["nc"]


def make_in_maps(query, Wq, bq, Wk, bk, Wv, bv, Wo, bo):
    import ml_dtypes

    f32 = np.float32
    bf16 = ml_dtypes.bfloat16
    query = np.asarray(query, f32)
    x3 = query.reshape(L, BSZ, E)  # faithful torch .view reshape

    Wq, Wk, Wv, Wo = (np.asarray(w, f32) for w in (Wq, Wk, Wv, Wo))
    bq, bk, bv = (np.asarray(b, f32) for b in (bq, bk, bv))

    def wslice_pair(W, p):
        w = W[P * p : P * (p + 1), :].T  # (512, 128)
        return np.hstack([w[e * P : (e + 1) * P, :] for e in range(4)])

    ident = np.eye(P, dtype=bf16)
    idx = (np.pi / 2) * np.arange(1, L + 1, dtype=f32) / L
    sin, cos = np.sin(idx), np.cos(idx)
    tt = np.arange(P)
    diff = tt[None, :] - tt[:, None]
    maskcos = np.where(diff >= 0, np.cos((np.pi / 2) * diff / L), 0.0).astype(f32)
    # sctab: (128, 16) bf16: per chunk c, col 2c = sin, col 2c+1 = cos
    sctab = np.zeros((P, 16), f32)
    for c in range(NCHUNK):
        sctab[:, 2 * c] = sin[c * P : (c + 1) * P]
        sctab[:, 2 * c + 1] = cos[c * P : (c + 1) * P]
    # scb rows: [sin; cos], scbf rows: [cos; sin]
    scb = np.empty((P, L), f32)
    scb[0:D] = sin[None, :]
    scb[D:P] = cos[None, :]
    scbf = np.empty((P, L), f32)
    scbf[0:D] = cos[None, :]
    scbf[D:P] = sin[None, :]

    in_maps = []
    for core in range(N_CORES):
        i, p = divmod(core, 4)

        bias_cols = np.ascontiguousarray(
            np.stack([bq[P * p : P * (p + 1)], bk[P * p : P * (p + 1)]], axis=1)
        ).view(bf16)  # (128, 2) f32 -> (128, 4) bf16 bits
        c1a = np.hstack(
            [
                bias_cols,
                wslice_pair(Wq, p).astype(bf16),
                wslice_pair(Wk, p).astype(bf16),
            ]
        )
        assert c1a.shape == (P, _C1A_COLS), c1a.shape

        c1b = np.hstack(
            [
                wslice_pair(Wv, p).astype(bf16),
                Wo[:, P * p : P * (p + 1)].T.astype(bf16),
                ident,
                maskcos.astype(bf16),
                sctab.astype(bf16),
            ]
        )
        assert c1b.shape == (P, _C1B_COLS), c1b.shape

        c3 = np.hstack([scb.astype(bf16), scbf.astype(bf16)])
        assert c3.shape == (P, _C3_COLS), c3.shape

        xt_full = np.ascontiguousarray(x3[:, i, :].T).astype(bf16)  # (512, 1024)
        blocks = [
            xt_full[128 * e : 128 * (e + 1), 512 * th : 512 * (th + 1)]
            for th in range(2)
            for e in range(4)
        ]
        xt = np.hstack(blocks)
        assert xt.shape == (P, 4096), xt.shape

        in_maps.append(
            dict(
                xt=np.ascontiguousarray(xt),
                c1a=np.ascontiguousarray(c1a),
                c1b=np.ascontiguousarray(c1b),
                c3=np.ascontiguousarray(c3),
            )
        )
    return in_maps


def assemble(partials, bo, bv, Wo):
    def unpack(arr):
        a = np.asarray(arr, np.float32).reshape(P, NCHUNK, E)
        return a.transpose(1, 0, 2).reshape(L, E)

    out_flat = np.zeros((BSZ * L, E), np.float32)
    out_flat[0::2] = sum(unpack(partials[j]) for j in range(4))
    out_flat[1::2] = sum(unpack(partials[j]) for j in range(4, 8))
    # attn(v + bv) = attn(v) + bv (exact up to eps clip), so fold bv@Wo.T into bo
    bo_eff = np.asarray(bo, np.float32) + np.asarray(bv, np.float32) @ np.asarray(
        Wo, np.float32
    ).T.astype(np.float32)
    out_flat += bo_eff[None, :]
    return out_flat.reshape(BSZ, L, E)


def run(inputs, trace=False):
    from concourse.bass_utils import run_bass_kernel_spmd

    in_maps = make_in_maps(**inputs)
    nc = _get_nc()
    res = run_bass_kernel_spmd(nc, in_maps, list(range(N_CORES)), trace=trace)
    partials = [r["out"] for r in res.results]
    return assemble(partials, inputs["bo"], inputs["bv"], inputs["Wo"]), res


def kernel(**inputs):
    out, _ = run(inputs, trace=False)
    return out


# revision 40
# speedup vs baseline: 1.2097x; 1.2097x over previous
"""Cosformer attention (causal linear attention with cos reweighting) on 8
Trainium2 NeuronCores.

Sharding: n = bsz*heads = 16 sequences -> 2 per core. Core c handles batch-half
i = c//4 and head-pair p = c%4 (heads 2p, 2p+1). Fully data/head parallel; the
only cross-core interaction is the host-side sum of output-projection partials.

v2 design notes:
  - Intra-chunk scores use RAW 64-dim q,k per head with the cos reweighting
    folded into the mask table: q_.k_ = q.k * cos(idx_t - idx_s), so
    maskcos[s,t] = triu * cos(pi/2 (t-s)/L). This kills the k feature-major
    expansion entirely and halves the score contraction depth.
  - sin/cos tables (scb/scbf for the q expansion, per-chunk sincol columns
    for the token-major k expansion) are host-precomputed and DMA'd: no
    iota, no ACT Sin, no Sin table load.
  - v is projected TOKEN-major (lhsT = x chunk, rhs = Wv) straight into the
    per-chunk (tokens, 65) layout, killing all 8 v transposes.
  - The serial S-state chain is replaced by 16 independent P_c = K_c^T V_c
    matmuls (4 chunks packed per bf16 PSUM bank) + a depth-3 prefix add tree
    on DVE, so the 8 attention chunk blocks are fully independent.
  - 5 input DMAs / 4 output DMAs total, big contiguous per-partition rows.
  - o-projection emits bf16 PSUM (single-group) for cheap packed evacuation,
    alternating ACT/DVE; outputs ship in (128,1024) pairs on the sync ring.
"""

import os
import sys

import numpy as np

for _p in ("/opt/trn_rl_repo", "/root/.axon_site/_ro/trn_rl_repo"):
    if os.path.isdir(_p) and _p not in sys.path:
        sys.path.insert(0, _p)

N_HEAD = 8
E = 512
L = 1024
BSZ = 2
D = 64
P = 128
NCHUNK = L // P
N_CORES = 8
TH = 512  # token-half width for q/k projections

# c1a: [bias 4 | wq 512 | wk 512] — only what the first projections need
_C1_BIAS = 0
_C1_WQ = 4
_C1_WK = 516
_C1A_COLS = 1028
# c1b: [wv 512 | wo 512 | ident 128 | maskcos 128 | sctab 16]
_C1B_WV = 0
_C1B_WO = 512
_C1B_IDENT = 1024
_C1B_MASK = 1152
_C1B_SCTAB = 1280
_C1B_COLS = 1296
# c3: [scb 1024 | scbf 1024]
_C3_COLS = 2048

_CACHE = {}


def _build_bass():
    import concourse.bass as bass
    import concourse.tile as tile
    from concourse import bacc, mybir
    from concourse.bass import _add_dep_helper
    from contextlib import ExitStack

    f32 = mybir.dt.float32
    bf16 = mybir.dt.bfloat16
    AF = mybir.ActivationFunctionType

    nc = bacc.Bacc("TRN2", target_bir_lowering=False, debug=False)

    xt_d = nc.dram_tensor("xt", [P, 4096], bf16, kind="ExternalInput")
    c1a_d = nc.dram_tensor("c1a", [P, _C1A_COLS], bf16, kind="ExternalInput")
    c1b_d = nc.dram_tensor("c1b", [P, _C1B_COLS], bf16, kind="ExternalInput")
    c3_d = nc.dram_tensor("c3", [P, _C3_COLS], bf16, kind="ExternalInput")
    out_d = nc.dram_tensor("out", [P, 4096], bf16, kind="ExternalOutput")

    with tile.TileContext(nc) as tc:
        with ExitStack() as ctx:
            ep = ctx.enter_context
            cpool = ep(tc.tile_pool(name="const", bufs=1))
            ktp = ep(tc.tile_pool(name="ktok", bufs=4))
            bp = ep(tc.tile_pool(name="bsb", bufs=3))
            pp = ep(tc.tile_pool(name="psb", bufs=4))
            spool = ep(tc.tile_pool(name="Ssb", bufs=10))
            ap_pool = ep(tc.tile_pool(name="apair", bufs=3))
            atp = ep(tc.tile_pool(name="attnT", bufs=3))
            outp = ep(tc.tile_pool(name="outsb", bufs=2))
            rp = ep(tc.tile_pool(name="rcol", bufs=4))
            big_ps = ep(tc.tile_pool(name="bigps", bufs=2, space="PSUM"))
            vs_ps = ep(tc.tile_pool(name="vsps", bufs=2, space="PSUM"))
            qkv_ps = ep(tc.tile_pool(name="qkvps", bufs=2, space="PSUM"))
            p_ps = ep(tc.tile_pool(name="pps", bufs=2, space="PSUM"))

            # ---- loads: scalar ring carries weights/tables, sync ring x ----
            c1a_t = cpool.tile([P, _C1A_COLS], bf16, name="c1a_t")
            c1b_t = cpool.tile([P, _C1B_COLS], bf16, name="c1b_t")
            c3_t = cpool.tile([P, _C3_COLS], bf16, name="c3_t")
            xt_sb = cpool.tile([P, 4096], bf16, name="xt_sb")
            # ONE strictly-ordered input queue (sync ring), critical bytes
            # first: the first projections gate on c1a+xt_e01 = 520KB only.
            # Outputs go on the scalar ring (no FIFO entanglement).
            dma0 = nc.sync.dma_start(c1a_t[:], c1a_d[:, :])
            nc.sync.dma_start(xt_sb[:, 0:1024], xt_d[:, 0:1024])
            nc.sync.dma_start(xt_sb[:, 1024:2048], xt_d[:, 1024:2048])
            nc.sync.dma_start(xt_sb[:, 2048:4096], xt_d[:, 2048:4096])
            nc.sync.dma_start(c1b_t[:], c1b_d[:, :])
            nc.sync.dma_start(c3_t[:], c3_d[:, :])

            wq = [c1a_t[:, _C1_WQ + e * P : _C1_WQ + (e + 1) * P] for e in range(4)]
            wk = [c1a_t[:, _C1_WK + e * P : _C1_WK + (e + 1) * P] for e in range(4)]
            wv = [c1b_t[:, _C1B_WV + e * P : _C1B_WV + (e + 1) * P] for e in range(4)]
            wo_t = c1b_t[:, _C1B_WO : _C1B_WO + 512]
            ident_t = c1b_t[:, _C1B_IDENT : _C1B_IDENT + P]
            maskcos_t = c1b_t[:, _C1B_MASK : _C1B_MASK + P]
            sctab = c1b_t[:, _C1B_SCTAB : _C1B_SCTAB + 16]  # (128,16) bf16 sin/cos
            bq_col = c1a_t[:, 0:2].bitcast(f32)
            bk_col = c1a_t[:, 2:4].bitcast(f32)
            scb = c3_t[:, 0:1024]
            scbf = c3_t[:, 1024:2048]

            def xslice(th, e):
                return xt_sb[:, 2048 * th + 512 * e : 2048 * th + 512 * (e + 1)]

            # ---- persistent activations ----
            # qz_a: head-a raw q on partitions 0:64, zeros elsewhere; qz_b:
            # head-b on 64:128, zeros elsewhere — so per-head scores can use
            # full-K=128 matmuls (legal same-bank packing, shared LDWEIGHTS).
            qz = {h: cpool.tile([P, L], bf16, name=f"qz_{h}") for h in "ab"}
            kp_sb = cpool.tile([P, L], bf16, name="kp_sb")
            q_seq = {h: cpool.tile([P, L], bf16, name=f"q_{h}") for h in "ab"}
            vt_all = cpool.tile([P, NCHUNK * 2 * (D + 1)], bf16, name="vt_all")
            # only the ones-columns need the memset (v cols get overwritten);
            # DVE, deferred past the first DMA so the exec-time window and the
            # early DMA stream stay clean.
            mset = nc.vector.memset(
                vt_all[:].rearrange("p (c e) -> p c e", c=2 * NCHUNK, e=65)[:, :, 64:65],
                1.0,
            )
            _add_dep_helper(mset.ins, dma0.ins, sync=True, reason="late memset")
            mza = nc.vector.memset(qz["a"][D:P, :], 0.0)
            _add_dep_helper(mza.ins, dma0.ins, sync=True, reason="late memset")
            mzb = nc.vector.memset(qz["b"][0:D, :], 0.0)
            _add_dep_helper(mzb.ins, dma0.ins, sync=True, reason="late memset")

            def proj_qk(th, what):
                sl = slice(th * TH, (th + 1) * TH)
                ps = big_ps.tile([P, TH], f32, tag="big", name=f"{what}_ps{th}")
                w = wq if what == "q" else wk
                for e in range(4):
                    nc.tensor.matmul(
                        ps[:], w[e], xslice(th, e), start=(e == 0), stop=(e == 3)
                    )
                if what == "q":
                    nc.scalar.activation(
                        qz["a"][0:D, sl], ps[0:D, :], AF.Relu, bias=bq_col[0:D, 0:1]
                    )
                    nc.scalar.activation(
                        qz["b"][D:P, sl], ps[D:P, :], AF.Relu, bias=bq_col[D:P, 0:1]
                    )
                else:
                    nc.scalar.activation(
                        kp_sb[:, sl], ps[:], AF.Relu, bias=bk_col[:, 0:1]
                    )

            def expand_q(th):
                sl = slice(th * TH, (th + 1) * TH)
                nc.vector.tensor_mul(q_seq["a"][0:D, sl], qz["a"][0:D, sl], scb[0:D, sl])
                nc.vector.tensor_mul(q_seq["a"][D:P, sl], qz["a"][0:D, sl], scbf[0:D, sl])
                nc.vector.tensor_mul(q_seq["b"][0:D, sl], qz["b"][D:P, sl], scbf[D:P, sl])
                nc.vector.tensor_mul(q_seq["b"][D:P, sl], qz["b"][D:P, sl], scb[D:P, sl])

            def proj_v_pair(c0):
                """Project chunks c0, c0+1 token-major into one PSUM bank."""
                v_ps = vs_ps.tile([P, 2 * P], f32, tag="vs", name=f"vps{c0}")
                for j in range(2):
                    th, lc = divmod(c0 + j, 4)
                    for e in range(4):
                        lhsT = xt_sb[
                            :,
                            2048 * th + 512 * e + P * lc : 2048 * th + 512 * e + P * (lc + 1),
                        ]
                        nc.tensor.matmul(
                            v_ps[:, j * P : (j + 1) * P], lhsT, wv[e],
                            start=(j == 0 and e == 0),
                            stop=(j == 1 and e == 3),
                            skip_group_check=True,
                        )
                nc.scalar.copy(
                    vt_all[:, c0 * 130 : (c0 + 2) * 130]
                    .rearrange("p (h e) -> p h e", h=4, e=65)[:, :, 0:64],
                    v_ps[:].rearrange("p (h e) -> p h e", h=4, e=64),
                )

            ktoks = {}

            def kt_chunk(c):
                cs = slice(c * P, (c + 1) * P)
                ktr = vs_ps.tile([P, P], bf16, tag="vs", name=f"ktr{c}")
                nc.tensor.matmul(ktr[:], kp_sb[:, cs], ident_t, is_transpose=True)
                ktok = ktp.tile([P, 2 * P], bf16, tag="ktok", name=f"ktok{c}")
                # ktok = [a*sin | a*cos | b*sin | b*cos] in ONE op: stride-0
                # read of ktr per (sin,cos), stride-0 bcast of the per-token
                # sin/cos column pair along d.
                nc.vector.tensor_mul(
                    ktok[:].rearrange("p (h s d) -> p h s d", h=2, s=2, d=64),
                    ktr[:].rearrange("p (h d) -> p h d", h=2, d=64)
                    .unsqueeze(2)
                    .broadcast_to([P, 2, 2, 64]),
                    sctab[:, 2 * c : 2 * c + 2]
                    .unsqueeze(1)
                    .unsqueeze(3)
                    .broadcast_to([P, 2, 2, 64]),
                )
                ktoks[c] = ktok

            def p_mms(cs2, name):
                """P_c = K_c_^T V_c_aug for 2 chunks into one f32 bank."""
                ps = p_ps.tile([P, 2 * 130], f32, tag="pp", name=name)
                n = len(cs2) * 2
                i = 0
                for j, c in enumerate(cs2):
                    for h in range(2):
                        # one start (bank has_written clear), one stop;
                        # skip_group_check everywhere so the sim's
                        # per-region group tracker stays out of the way.
                        nc.tensor.matmul(
                            ps[:, j * 130 + h * 65 : j * 130 + (h + 1) * 65],
                            ktoks[c][:, h * P : (h + 1) * P],
                            vt_all[:, c * 130 + h * 65 : c * 130 + (h + 1) * 65],
                            start=(i == 0),
                            stop=(i == n - 1),
                            skip_group_check=True,
                        )
                        i += 1
                return ps

            def sadd(name, a, b, eng=None):
                t = spool.tile([P, 130], bf16, tag="S", name=name)
                (eng or nc.vector).tensor_add(t[:], a, b)
                return t

            o_sbs = [
                outp.tile([P, 2 * E], bf16, tag="osb", name=f"osb{i}") for i in range(2)
            ]

            def attn_chunk(c, S_prev):
                cs = slice(c * P, (c + 1) * P)
                sc = vs_ps.tile([P, 2 * P], f32, tag="vs", name=f"sc{c}")
                nc.tensor.matmul(
                    sc[:, 0:P], kp_sb[:, cs], qz["a"][:, cs],
                    start=True, stop=False, skip_group_check=True,
                )
                nc.tensor.matmul(
                    sc[:, P : 2 * P], kp_sb[:, cs], qz["b"][:, cs],
                    start=False, stop=True, skip_group_check=True,
                )
                b_sb = bp.tile([P, 2 * P], bf16, tag="bsb", name=f"bsb{c}")
                nc.vector.tensor_mul(
                    b_sb[:].rearrange("p (h t) -> p h t", h=2, t=P),
                    sc[:].rearrange("p (h t) -> p h t", h=2, t=P),
                    maskcos_t.unsqueeze(1).broadcast_to([P, 2, P]),
                )
                qkv = qkv_ps.tile([P, 2 * (D + 1)], f32, tag="qkv", name=f"qkv{c}")
                nc.tensor.matmul(
                    qkv[:, 0:65], b_sb[:, 0:P], vt_all[:, c * 130 : c * 130 + 65],
                    start=True, stop=False, skip_group_check=True,
                )
                nc.tensor.matmul(
                    qkv[:, 65:130], b_sb[:, P : 2 * P],
                    vt_all[:, c * 130 + 65 : (c + 1) * 130],
                    start=False, stop=(c == 0), skip_group_check=True,
                )
                if c > 0:
                    nc.tensor.matmul(
                        qkv[:, 0:65], q_seq["a"][:, cs], S_prev[:, 0:65],
                        start=False, stop=False, skip_group_check=True,
                    )
                    nc.tensor.matmul(
                        qkv[:, 65:130], q_seq["b"][:, cs], S_prev[:, 65:130],
                        start=False, stop=True, skip_group_check=True,
                    )
                r_col = rp.tile([P, 4], f32, tag="r", name=f"r{c}")
                nc.vector.reciprocal(r_col[:, 2:4], qkv[:, 64:130:65])
                attn_pair = ap_pool.tile([P, P], bf16, tag="ap", name=f"ap{c}")
                nc.vector.tensor_mul(
                    attn_pair[:].rearrange("p (h f) -> p h f", h=2, f=64),
                    qkv[:].rearrange("p (h f) -> p h f", h=2, f=65)[:, :, 0:64],
                    r_col[:, 2:4].unsqueeze(2).broadcast_to([P, 2, 64]),
                )
                at_ps = p_ps.tile([P, P], bf16, tag="pp", name=f"atps{c}")
                nc.tensor.matmul(at_ps[:], attn_pair[:], ident_t, is_transpose=True)
                at_sb = atp.tile([P, P], bf16, tag="at", name=f"at{c}")
                nc.vector.tensor_copy(at_sb[:], at_ps[:])
                o_ps = big_ps.tile([P, E], f32, tag="big", name=f"ops{c}")
                nc.tensor.matmul(o_ps[:], at_sb[:], wo_t, start=True, stop=True)
                o_sb = o_sbs[(c // 2) % 2]
                osl = o_sb[:, (c % 2) * E : (c % 2 + 1) * E]
                nc.scalar.copy(osl, o_ps[:])
                if c >= 6:
                    # tail chunks ship individually so the last transfer is
                    # small and the drain phase starts early
                    nc.scalar.dma_start(out_d[:, c * E : (c + 1) * E], osl)
                elif c % 2 == 1:
                    nc.scalar.dma_start(out_d[:, (c - 1) * E : (c + 1) * E], o_sb[:])

            # ---- program order (== scheduler priority) ----
            proj_qk(0, "q")
            expand_q(0)
            proj_qk(0, "k")
            proj_v_pair(0)
            proj_v_pair(2)
            proj_qk(1, "q")
            expand_q(1)
            proj_qk(1, "k")
            proj_v_pair(4)
            proj_v_pair(6)
            for c in range(4):
                kt_chunk(c)
            ps01 = p_mms((0, 1), "P01ps")
            p01 = pp.tile([P, 260], bf16, tag="P", name="P01")
            nc.scalar.copy(p01[:], ps01[:])
            ps23 = p_mms((2, 3), "P23ps")
            p23 = pp.tile([P, 260], bf16, tag="P", name="P23")
            nc.scalar.copy(p23[:], ps23[:])
            for c in range(4, 8):
                kt_chunk(c)
            ps45 = p_mms((4, 5), "P45ps")
            p45 = pp.tile([P, 260], bf16, tag="P", name="P45")
            nc.scalar.copy(p45[:], ps45[:])
            ps67 = p_mms((6, 7), "P67ps")
            p67 = pp.tile([P, 260], bf16, tag="P", name="P67")
            nc.scalar.copy(p67[:], ps67[:])

            # prefix tree: S_c = sum_{j<=c} P_j  (bf16 adds, depth <= 3)
            s1 = sadd("S1", p01[:, 0:130], p01[:, 130:260])
            a23 = sadd("A23", p23[:, 0:130], p23[:, 130:260], nc.gpsimd)
            s2 = sadd("S2", s1[:], p23[:, 0:130])
            s3 = sadd("S3", s1[:], a23[:])
            a45 = sadd("A45", p45[:, 0:130], p45[:, 130:260], nc.gpsimd)
            a456 = sadd("A456", a45[:], p67[:, 0:130], nc.gpsimd)
            s4 = sadd("S4", s3[:], p45[:, 0:130])
            s5 = sadd("S5", s3[:], a45[:])
            s6 = sadd("S6", s3[:], a456[:])
            S_list = [p01[:, 0:130], s1[:], s2[:], s3[:], s4[:], s5[:], s6[:]]

            attn_chunk(0, None)
            for c in range(1, NCHUNK):
                attn_chunk(c, S_list[c - 1])

    nc.compile()
    return nc


def _get_nc():
    if "nc" not in _CACHE:
        _CACHE["nc"] = _build_bass()
    return _CACHE["nc"]


def make_in_maps(query, Wq, bq, Wk, bk, Wv, bv, Wo, bo):
    import ml_dtypes

    f32 = np.float32
    bf16 = ml_dtypes.bfloat16
    query = np.asarray(query, f32)
    x3 = query.reshape(L, BSZ, E)  # faithful torch .view reshape

    Wq, Wk, Wv, Wo = (np.asarray(w, f32) for w in (Wq, Wk, Wv, Wo))
    bq, bk, bv = (np.asarray(b, f32) for b in (bq, bk, bv))

    def wslice_pair(W, p):
        w = W[P * p : P * (p + 1), :].T  # (512, 128)
        return np.hstack([w[e * P : (e + 1) * P, :] for e in range(4)])

    ident = np.eye(P, dtype=bf16)
    idx = (np.pi / 2) * np.arange(1, L + 1, dtype=f32) / L
    sin, cos = np.sin(idx), np.cos(idx)
    tt = np.arange(P)
    diff = tt[None, :] - tt[:, None]
    maskcos = np.where(diff >= 0, np.cos((np.pi / 2) * diff / L), 0.0).astype(f32)
    # sctab: (128, 16) bf16: per chunk c, col 2c = sin, col 2c+1 = cos
    sctab = np.zeros((P, 16), f32)
    for c in range(NCHUNK):
        sctab[:, 2 * c] = sin[c * P : (c + 1) * P]
        sctab[:, 2 * c + 1] = cos[c * P : (c + 1) * P]
    # scb rows: [sin; cos], scbf rows: [cos; sin]
    scb = np.empty((P, L), f32)
    scb[0:D] = sin[None, :]
    scb[D:P] = cos[None, :]
    scbf = np.empty((P, L), f32)
    scbf[0:D] = cos[None, :]
    scbf[D:P] = sin[None, :]

    in_maps = []
    for core in range(N_CORES):
        i, p = divmod(core, 4)

        bias_cols = np.ascontiguousarray(
            np.stack([bq[P * p : P * (p + 1)], bk[P * p : P * (p + 1)]], axis=1)
        ).view(bf16)  # (128, 2) f32 -> (128, 4) bf16 bits
        c1a = np.hstack(
            [
                bias_cols,
                wslice_pair(Wq, p).astype(bf16),
                wslice_pair(Wk, p).astype(bf16),
            ]
        )
        assert c1a.shape == (P, _C1A_COLS), c1a.shape

        c1b = np.hstack(
            [
                wslice_pair(Wv, p).astype(bf16),
                Wo[:, P * p : P * (p + 1)].T.astype(bf16),
                ident,
                maskcos.astype(bf16),
                sctab.astype(bf16),
            ]
        )
        assert c1b.shape == (P, _C1B_COLS), c1b.shape

        c3 = np.hstack([scb.astype(bf16), scbf.astype(bf16)])
        assert c3.shape == (P, _C3_COLS), c3.shape

        xt_full = np.ascontiguousarray(x3[:, i, :].T).astype(bf16)  # (512, 1024)
        blocks = [
            xt_full[128 * e : 128 * (e + 1), 512 * th : 512 * (th + 1)]
            for th in range(2)
            for e in range(4)
        ]
        xt = np.hstack(blocks)
        assert xt.shape == (P, 4096), xt.shape

        in_maps.append(
            dict(
                xt=np.ascontiguousarray(xt),
                c1a=np.ascontiguousarray(c1a),
                c1b=np.ascontiguousarray(c1b),
                c3=np.ascontiguousarray(c3),
            )
        )
    return in_maps


def assemble(partials, bo, bv, Wo):
    def unpack(arr):
        a = np.asarray(arr, np.float32).reshape(P, NCHUNK, E)
        return a.transpose(1, 0, 2).reshape(L, E)

    out_flat = np.zeros((BSZ * L, E), np.float32)
    out_flat[0::2] = sum(unpack(partials[j]) for j in range(4))
    out_flat[1::2] = sum(unpack(partials[j]) for j in range(4, 8))
    # attn(v + bv) = attn(v) + bv (exact up to eps clip), so fold bv@Wo.T into bo
    bo_eff = np.asarray(bo, np.float32) + np.asarray(bv, np.float32) @ np.asarray(
        Wo, np.float32
    ).T.astype(np.float32)
    out_flat += bo_eff[None, :]
    return out_flat.reshape(BSZ, L, E)


def run(inputs, trace=False):
    from concourse.bass_utils import run_bass_kernel_spmd

    in_maps = make_in_maps(**inputs)
    nc = _get_nc()
    res = run_bass_kernel_spmd(nc, in_maps, list(range(N_CORES)), trace=trace)
    partials = [r["out"] for r in res.results]
    return assemble(partials, inputs["bo"], inputs["bv"], inputs["Wo"]), res


def kernel(**inputs):
    out, _ = run(inputs, trace=False)
    return out


# revision 42
# speedup vs baseline: 1.2114x; 1.0014x over previous
"""Cosformer attention (causal linear attention with cos reweighting) on 8
Trainium2 NeuronCores.

Sharding: n = bsz*heads = 16 sequences -> 2 per core. Core c handles batch-half
i = c//4 and head-pair p = c%4 (heads 2p, 2p+1). Fully data/head parallel; the
only cross-core interaction is the host-side sum of output-projection partials.

v2 design notes:
  - Intra-chunk scores use RAW 64-dim q,k per head with the cos reweighting
    folded into the mask table: q_.k_ = q.k * cos(idx_t - idx_s), so
    maskcos[s,t] = triu * cos(pi/2 (t-s)/L). This kills the k feature-major
    expansion entirely and halves the score contraction depth.
  - sin/cos tables (scb/scbf for the q expansion, per-chunk sincol columns
    for the token-major k expansion) are host-precomputed and DMA'd: no
    iota, no ACT Sin, no Sin table load.
  - v is projected TOKEN-major (lhsT = x chunk, rhs = Wv) straight into the
    per-chunk (tokens, 65) layout, killing all 8 v transposes.
  - The serial S-state chain is replaced by 16 independent P_c = K_c^T V_c
    matmuls (4 chunks packed per bf16 PSUM bank) + a depth-3 prefix add tree
    on DVE, so the 8 attention chunk blocks are fully independent.
  - 5 input DMAs / 4 output DMAs total, big contiguous per-partition rows.
  - o-projection emits bf16 PSUM (single-group) for cheap packed evacuation,
    alternating ACT/DVE; outputs ship in (128,1024) pairs on the sync ring.
"""

import os
import sys

import numpy as np

for _p in ("/opt/trn_rl_repo", "/root/.axon_site/_ro/trn_rl_repo"):
    if os.path.isdir(_p) and _p not in sys.path:
        sys.path.insert(0, _p)

N_HEAD = 8
E = 512
L = 1024
BSZ = 2
D = 64
P = 128
NCHUNK = L // P
N_CORES = 8
TH = 512  # token-half width for q/k projections

# c1a: [bias 4 | wq 512 | wk 512] — only what the first projections need
_C1_BIAS = 0
_C1_WQ = 4
_C1_WK = 516
_C1A_COLS = 1028
# c1b: [wv 512 | wo 512 | ident 128 | maskcos 128 | sctab 16]
_C1B_WV = 0
_C1B_WO = 512
_C1B_IDENT = 1024
_C1B_MASK = 1152
_C1B_SCTAB = 1280
_C1B_COLS = 1296
# c3: [scb 1024 | scbf 1024]
_C3_COLS = 2048

_CACHE = {}


def _build_bass():
    import concourse.bass as bass
    import concourse.tile as tile
    from concourse import bacc, mybir
    from concourse.bass import _add_dep_helper
    from contextlib import ExitStack

    f32 = mybir.dt.float32
    bf16 = mybir.dt.bfloat16
    AF = mybir.ActivationFunctionType

    nc = bacc.Bacc("TRN2", target_bir_lowering=False, debug=False)

    xt_d = nc.dram_tensor("xt", [P, 4096], bf16, kind="ExternalInput")
    c1a_d = nc.dram_tensor("c1a", [P, _C1A_COLS], bf16, kind="ExternalInput")
    c1b_d = nc.dram_tensor("c1b", [P, _C1B_COLS], bf16, kind="ExternalInput")
    c3_d = nc.dram_tensor("c3", [P, _C3_COLS], bf16, kind="ExternalInput")
    out_d = nc.dram_tensor("out", [P, 4096], bf16, kind="ExternalOutput")

    with tile.TileContext(nc) as tc:
        with ExitStack() as ctx:
            ep = ctx.enter_context
            cpool = ep(tc.tile_pool(name="const", bufs=1))
            ktp = ep(tc.tile_pool(name="ktok", bufs=4))
            bp = ep(tc.tile_pool(name="bsb", bufs=3))
            pp = ep(tc.tile_pool(name="psb", bufs=4))
            spool = ep(tc.tile_pool(name="Ssb", bufs=10))
            ap_pool = ep(tc.tile_pool(name="apair", bufs=3))
            atp = ep(tc.tile_pool(name="attnT", bufs=3))
            outp = ep(tc.tile_pool(name="outsb", bufs=2))
            rp = ep(tc.tile_pool(name="rcol", bufs=4))
            big_ps = ep(tc.tile_pool(name="bigps", bufs=2, space="PSUM"))
            vs_ps = ep(tc.tile_pool(name="vsps", bufs=2, space="PSUM"))
            qkv_ps = ep(tc.tile_pool(name="qkvps", bufs=2, space="PSUM"))
            p_ps = ep(tc.tile_pool(name="pps", bufs=2, space="PSUM"))

            # ---- loads: scalar ring carries weights/tables, sync ring x ----
            c1a_t = cpool.tile([P, _C1A_COLS], bf16, name="c1a_t")
            c1b_t = cpool.tile([P, _C1B_COLS], bf16, name="c1b_t")
            c3_t = cpool.tile([P, _C3_COLS], bf16, name="c3_t")
            xt_sb = cpool.tile([P, 4096], bf16, name="xt_sb")
            # ONE strictly-ordered input queue (sync ring), critical bytes
            # first: the first projections gate on c1a+xt_e01 = 520KB only.
            # Outputs go on the scalar ring (no FIFO entanglement).
            dma0 = nc.sync.dma_start(c1a_t[:], c1a_d[:, :])
            nc.sync.dma_start(xt_sb[:, 0:1024], xt_d[:, 0:1024])
            nc.sync.dma_start(xt_sb[:, 1024:2048], xt_d[:, 1024:2048])
            nc.sync.dma_start(xt_sb[:, 2048:4096], xt_d[:, 2048:4096])
            nc.sync.dma_start(c1b_t[:], c1b_d[:, :])
            nc.sync.dma_start(c3_t[:], c3_d[:, :])

            wq = [c1a_t[:, _C1_WQ + e * P : _C1_WQ + (e + 1) * P] for e in range(4)]
            wk = [c1a_t[:, _C1_WK + e * P : _C1_WK + (e + 1) * P] for e in range(4)]
            wv = [c1b_t[:, _C1B_WV + e * P : _C1B_WV + (e + 1) * P] for e in range(4)]
            wo_t = c1b_t[:, _C1B_WO : _C1B_WO + 512]
            ident_t = c1b_t[:, _C1B_IDENT : _C1B_IDENT + P]
            maskcos_t = c1b_t[:, _C1B_MASK : _C1B_MASK + P]
            sctab = c1b_t[:, _C1B_SCTAB : _C1B_SCTAB + 16]  # (128,16) bf16 sin/cos
            bq_col = c1a_t[:, 0:2].bitcast(f32)
            bk_col = c1a_t[:, 2:4].bitcast(f32)
            scb = c3_t[:, 0:1024]
            scbf = c3_t[:, 1024:2048]

            def xslice(th, e):
                return xt_sb[:, 2048 * th + 512 * e : 2048 * th + 512 * (e + 1)]

            # ---- persistent activations ----
            # qz_a: head-a raw q on partitions 0:64, zeros elsewhere; qz_b:
            # head-b on 64:128, zeros elsewhere — so per-head scores can use
            # full-K=128 matmuls (legal same-bank packing, shared LDWEIGHTS).
            qz = {h: cpool.tile([P, L], bf16, name=f"qz_{h}") for h in "ab"}
            kp_sb = cpool.tile([P, L], bf16, name="kp_sb")
            q_seq = {h: cpool.tile([P, L], bf16, name=f"q_{h}") for h in "ab"}
            vt_all = cpool.tile([P, NCHUNK * 2 * (D + 1)], bf16, name="vt_all")
            # only the ones-columns need the memset (v cols get overwritten);
            # DVE, deferred past the first DMA so the exec-time window and the
            # early DMA stream stay clean.
            mset = nc.vector.memset(
                vt_all[:].rearrange("p (c e) -> p c e", c=2 * NCHUNK, e=65)[:, :, 64:65],
                1.0,
            )
            _add_dep_helper(mset.ins, dma0.ins, sync=True, reason="late memset")
            mza = nc.vector.memset(qz["a"][D:P, :], 0.0)
            _add_dep_helper(mza.ins, dma0.ins, sync=True, reason="late memset")
            mzb = nc.vector.memset(qz["b"][0:D, :], 0.0)
            _add_dep_helper(mzb.ins, dma0.ins, sync=True, reason="late memset")

            # ---- HAM warm-up: dependency-free matmuls on scratch SBUF fill
            # the PE-idle window during the input DMA (t~7-12.4us) so the PE
            # clock gate is at 8/8 (2.4 GHz) when the real projections start.
            # Results are never read; the scratch tile is never written.
            warm_sb = cpool.tile([P, 640], bf16, name="warm_sb")
            nc.vector.memset(warm_sb[:], 1.0)  # dep-free, runs pre-DMA
            warm_ps = big_ps.tile([P, TH], f32, tag="big", name="warm_ps")
            for i in range(12):
                nc.tensor.matmul(
                    warm_ps[:], warm_sb[:, 512:640], warm_sb[:, 0:512],
                    start=(i == 0), stop=(i == 11),
                )

            def proj_qk(th, what):
                sl = slice(th * TH, (th + 1) * TH)
                ps = big_ps.tile([P, TH], f32, tag="big", name=f"{what}_ps{th}")
                w = wq if what == "q" else wk
                for e in range(4):
                    nc.tensor.matmul(
                        ps[:], w[e], xslice(th, e), start=(e == 0), stop=(e == 3)
                    )
                if what == "q":
                    nc.scalar.activation(
                        qz["a"][0:D, sl], ps[0:D, :], AF.Relu, bias=bq_col[0:D, 0:1]
                    )
                    nc.scalar.activation(
                        qz["b"][D:P, sl], ps[D:P, :], AF.Relu, bias=bq_col[D:P, 0:1]
                    )
                else:
                    nc.scalar.activation(
                        kp_sb[:, sl], ps[:], AF.Relu, bias=bk_col[:, 0:1]
                    )

            def expand_q(th):
                sl = slice(th * TH, (th + 1) * TH)
                nc.vector.tensor_mul(q_seq["a"][0:D, sl], qz["a"][0:D, sl], scb[0:D, sl])
                nc.vector.tensor_mul(q_seq["a"][D:P, sl], qz["a"][0:D, sl], scbf[0:D, sl])
                nc.vector.tensor_mul(q_seq["b"][0:D, sl], qz["b"][D:P, sl], scbf[D:P, sl])
                nc.vector.tensor_mul(q_seq["b"][D:P, sl], qz["b"][D:P, sl], scb[D:P, sl])

            def proj_v_pair(c0):
                """Project chunks c0, c0+1 token-major into one PSUM bank."""
                v_ps = vs_ps.tile([P, 2 * P], f32, tag="vs", name=f"vps{c0}")
                for j in range(2):
                    th, lc = divmod(c0 + j, 4)
                    for e in range(4):
                        lhsT = xt_sb[
                            :,
                            2048 * th + 512 * e + P * lc : 2048 * th + 512 * e + P * (lc + 1),
                        ]
                        nc.tensor.matmul(
                            v_ps[:, j * P : (j + 1) * P], lhsT, wv[e],
                            start=(j == 0 and e == 0),
                            stop=(j == 1 and e == 3),
                            skip_group_check=True,
                        )
                nc.scalar.copy(
                    vt_all[:, c0 * 130 : (c0 + 2) * 130]
                    .rearrange("p (h e) -> p h e", h=4, e=65)[:, :, 0:64],
                    v_ps[:].rearrange("p (h e) -> p h e", h=4, e=64),
                )

            ktoks = {}

            def kt_chunk(c):
                cs = slice(c * P, (c + 1) * P)
                ktr = vs_ps.tile([P, P], bf16, tag="vs", name=f"ktr{c}")
                nc.tensor.matmul(ktr[:], kp_sb[:, cs], ident_t, is_transpose=True)
                ktok = ktp.tile([P, 2 * P], bf16, tag="ktok", name=f"ktok{c}")
                # ktok = [a*sin | a*cos | b*sin | b*cos] in ONE op: stride-0
                # read of ktr per (sin,cos), stride-0 bcast of the per-token
                # sin/cos column pair along d.
                nc.vector.tensor_mul(
                    ktok[:].rearrange("p (h s d) -> p h s d", h=2, s=2, d=64),
                    ktr[:].rearrange("p (h d) -> p h d", h=2, d=64)
                    .unsqueeze(2)
                    .broadcast_to([P, 2, 2, 64]),
                    sctab[:, 2 * c : 2 * c + 2]
                    .unsqueeze(1)
                    .unsqueeze(3)
                    .broadcast_to([P, 2, 2, 64]),
                )
                ktoks[c] = ktok

            def p_mms(cs2, name):
                """P_c = K_c_^T V_c_aug for 2 chunks into one f32 bank."""
                ps = p_ps.tile([P, 2 * 130], f32, tag="pp", name=name)
                n = len(cs2) * 2
                i = 0
                for j, c in enumerate(cs2):
                    for h in range(2):
                        # one start (bank has_written clear), one stop;
                        # skip_group_check everywhere so the sim's
                        # per-region group tracker stays out of the way.
                        nc.tensor.matmul(
                            ps[:, j * 130 + h * 65 : j * 130 + (h + 1) * 65],
                            ktoks[c][:, h * P : (h + 1) * P],
                            vt_all[:, c * 130 + h * 65 : c * 130 + (h + 1) * 65],
                            start=(i == 0),
                            stop=(i == n - 1),
                            skip_group_check=True,
                        )
                        i += 1
                return ps

            def sadd(name, a, b, eng=None):
                t = spool.tile([P, 130], bf16, tag="S", name=name)
                (eng or nc.vector).tensor_add(t[:], a, b)
                return t

            o_sbs = [
                outp.tile([P, 2 * E], bf16, tag="osb", name=f"osb{i}") for i in range(2)
            ]

            def attn_chunk(c, S_prev):
                cs = slice(c * P, (c + 1) * P)
                sc = vs_ps.tile([P, 2 * P], f32, tag="vs", name=f"sc{c}")
                nc.tensor.matmul(
                    sc[:, 0:P], kp_sb[:, cs], qz["a"][:, cs],
                    start=True, stop=False, skip_group_check=True,
                )
                nc.tensor.matmul(
                    sc[:, P : 2 * P], kp_sb[:, cs], qz["b"][:, cs],
                    start=False, stop=True, skip_group_check=True,
                )
                b_sb = bp.tile([P, 2 * P], bf16, tag="bsb", name=f"bsb{c}")
                nc.vector.tensor_mul(
                    b_sb[:].rearrange("p (h t) -> p h t", h=2, t=P),
                    sc[:].rearrange("p (h t) -> p h t", h=2, t=P),
                    maskcos_t.unsqueeze(1).broadcast_to([P, 2, P]),
                )
                qkv = qkv_ps.tile([P, 2 * (D + 1)], f32, tag="qkv", name=f"qkv{c}")
                nc.tensor.matmul(
                    qkv[:, 0:65], b_sb[:, 0:P], vt_all[:, c * 130 : c * 130 + 65],
                    start=True, stop=False, skip_group_check=True,
                )
                nc.tensor.matmul(
                    qkv[:, 65:130], b_sb[:, P : 2 * P],
                    vt_all[:, c * 130 + 65 : (c + 1) * 130],
                    start=False, stop=(c == 0), skip_group_check=True,
                )
                if c > 0:
                    nc.tensor.matmul(
                        qkv[:, 0:65], q_seq["a"][:, cs], S_prev[:, 0:65],
                        start=False, stop=False, skip_group_check=True,
                    )
                    nc.tensor.matmul(
                        qkv[:, 65:130], q_seq["b"][:, cs], S_prev[:, 65:130],
                        start=False, stop=True, skip_group_check=True,
                    )
                r_col = rp.tile([P, 4], f32, tag="r", name=f"r{c}")
                nc.vector.reciprocal(r_col[:, 2:4], qkv[:, 64:130:65])
                attn_pair = ap_pool.tile([P, P], bf16, tag="ap", name=f"ap{c}")
                nc.vector.tensor_mul(
                    attn_pair[:].rearrange("p (h f) -> p h f", h=2, f=64),
                    qkv[:].rearrange("p (h f) -> p h f", h=2, f=65)[:, :, 0:64],
                    r_col[:, 2:4].unsqueeze(2).broadcast_to([P, 2, 64]),
                )
                at_ps = p_ps.tile([P, P], bf16, tag="pp", name=f"atps{c}")
                nc.tensor.matmul(at_ps[:], attn_pair[:], ident_t, is_transpose=True)
                at_sb = atp.tile([P, P], bf16, tag="at", name=f"at{c}")
                nc.vector.tensor_copy(at_sb[:], at_ps[:])
                o_ps = big_ps.tile([P, E], f32, tag="big", name=f"ops{c}")
                nc.tensor.matmul(o_ps[:], at_sb[:], wo_t, start=True, stop=True)
                o_sb = o_sbs[(c // 2) % 2]
                osl = o_sb[:, (c % 2) * E : (c % 2 + 1) * E]
                nc.scalar.copy(osl, o_ps[:])
                if c >= 6:
                    # tail chunks ship individually so the last transfer is
                    # small and the drain phase starts early
                    nc.scalar.dma_start(out_d[:, c * E : (c + 1) * E], osl)
                elif c % 2 == 1:
                    nc.scalar.dma_start(out_d[:, (c - 1) * E : (c + 1) * E], o_sb[:])

            # ---- program order (== scheduler priority) ----
            proj_qk(0, "q")
            expand_q(0)
            proj_qk(0, "k")
            proj_v_pair(0)
            proj_v_pair(2)
            proj_qk(1, "q")
            expand_q(1)
            proj_qk(1, "k")
            proj_v_pair(4)
            proj_v_pair(6)
            for c in range(4):
                kt_chunk(c)
            ps01 = p_mms((0, 1), "P01ps")
            p01 = pp.tile([P, 260], bf16, tag="P", name="P01")
            nc.scalar.copy(p01[:], ps01[:])
            ps23 = p_mms((2, 3), "P23ps")
            p23 = pp.tile([P, 260], bf16, tag="P", name="P23")
            nc.scalar.copy(p23[:], ps23[:])
            for c in range(4, 8):
                kt_chunk(c)
            ps45 = p_mms((4, 5), "P45ps")
            p45 = pp.tile([P, 260], bf16, tag="P", name="P45")
            nc.scalar.copy(p45[:], ps45[:])
            ps67 = p_mms((6, 7), "P67ps")
            p67 = pp.tile([P, 260], bf16, tag="P", name="P67")
            nc.scalar.copy(p67[:], ps67[:])

            # prefix tree: S_c = sum_{j<=c} P_j  (bf16 adds, depth <= 3)
            s1 = sadd("S1", p01[:, 0:130], p01[:, 130:260])
            a23 = sadd("A23", p23[:, 0:130], p23[:, 130:260], nc.gpsimd)
            s2 = sadd("S2", s1[:], p23[:, 0:130])
            s3 = sadd("S3", s1[:], a23[:])
            a45 = sadd("A45", p45[:, 0:130], p45[:, 130:260], nc.gpsimd)
            a456 = sadd("A456", a45[:], p67[:, 0:130], nc.gpsimd)
            s4 = sadd("S4", s3[:], p45[:, 0:130])
            s5 = sadd("S5", s3[:], a45[:])
            s6 = sadd("S6", s3[:], a456[:])
            S_list = [p01[:, 0:130], s1[:], s2[:], s3[:], s4[:], s5[:], s6[:]]

            attn_chunk(0, None)
            for c in range(1, NCHUNK):
                attn_chunk(c, S_list[c - 1])

    nc.compile()
    return nc


def _get_nc():
    if "nc" not in _CACHE:
        _CACHE["nc"] = _build_bass()
    return _CACHE["nc"]


def make_in_maps(query, Wq, bq, Wk, bk, Wv, bv, Wo, bo):
    import ml_dtypes

    f32 = np.float32
    bf16 = ml_dtypes.bfloat16
    query = np.asarray(query, f32)
    x3 = query.reshape(L, BSZ, E)  # faithful torch .view reshape

    Wq, Wk, Wv, Wo = (np.asarray(w, f32) for w in (Wq, Wk, Wv, Wo))
    bq, bk, bv = (np.asarray(b, f32) for b in (bq, bk, bv))

    def wslice_pair(W, p):
        w = W[P * p : P * (p + 1), :].T  # (512, 128)
        return np.hstack([w[e * P : (e + 1) * P, :] for e in range(4)])

    ident = np.eye(P, dtype=bf16)
    idx = (np.pi / 2) * np.arange(1, L + 1, dtype=f32) / L
    sin, cos = np.sin(idx), np.cos(idx)
    tt = np.arange(P)
    diff = tt[None, :] - tt[:, None]
    maskcos = np.where(diff >= 0, np.cos((np.pi / 2) * diff / L), 0.0).astype(f32)
    # sctab: (128, 16) bf16: per chunk c, col 2c = sin, col 2c+1 = cos
    sctab = np.zeros((P, 16), f32)
    for c in range(NCHUNK):
        sctab[:, 2 * c] = sin[c * P : (c + 1) * P]
        sctab[:, 2 * c + 1] = cos[c * P : (c + 1) * P]
    # scb rows: [sin; cos], scbf rows: [cos; sin]
    scb = np.empty((P, L), f32)
    scb[0:D] = sin[None, :]
    scb[D:P] = cos[None, :]
    scbf = np.empty((P, L), f32)
    scbf[0:D] = cos[None, :]
    scbf[D:P] = sin[None, :]

    in_maps = []
    for core in range(N_CORES):
        i, p = divmod(core, 4)

        bias_cols = np.ascontiguousarray(
            np.stack([bq[P * p : P * (p + 1)], bk[P * p : P * (p + 1)]], axis=1)
        ).view(bf16)  # (128, 2) f32 -> (128, 4) bf16 bits
        c1a = np.hstack(
            [
                bias_cols,
                wslice_pair(Wq, p).astype(bf16),
                wslice_pair(Wk, p).astype(bf16),
            ]
        )
        assert c1a.shape == (P, _C1A_COLS), c1a.shape

        c1b = np.hstack(
            [
                wslice_pair(Wv, p).astype(bf16),
                Wo[:, P * p : P * (p + 1)].T.astype(bf16),
                ident,
                maskcos.astype(bf16),
                sctab.astype(bf16),
            ]
        )
        assert c1b.shape == (P, _C1B_COLS), c1b.shape

        c3 = np.hstack([scb.astype(bf16), scbf.astype(bf16)])
        assert c3.shape == (P, _C3_COLS), c3.shape

        xt_full = np.ascontiguousarray(x3[:, i, :].T).astype(bf16)  # (512, 1024)
        blocks = [
            xt_full[128 * e : 128 * (e + 1), 512 * th : 512 * (th + 1)]
            for th in range(2)
            for e in range(4)
        ]
        xt = np.hstack(blocks)
        assert xt.shape == (P, 4096), xt.shape

        in_maps.append(
            dict(
                xt=np.ascontiguousarray(xt),
                c1a=np.ascontiguousarray(c1a),
                c1b=np.ascontiguousarray(c1b),
                c3=np.ascontiguousarray(c3),
            )
        )
    return in_maps


def assemble(partials, bo, bv, Wo):
    def unpack(arr):
        a = np.asarray(arr, np.float32).reshape(P, NCHUNK, E)
        return a.transpose(1, 0, 2).reshape(L, E)

    out_flat = np.zeros((BSZ * L, E), np.float32)
    out_flat[0::2] = sum(unpack(partials[j]) for j in range(4))
    out_flat[1::2] = sum(unpack(partials[j]) for j in range(4, 8))
    # attn(v + bv) = attn(v) + bv (exact up to eps clip), so fold bv@Wo.T into bo
    bo_eff = np.asarray(bo, np.float32) + np.asarray(bv, np.float32) @ np.asarray(
        Wo, np.float32
    ).T.astype(np.float32)
    out_flat += bo_eff[None, :]
    return out_flat.reshape(BSZ, L, E)


def run(inputs, trace=False):
    from concourse.bass_utils import run_bass_kernel_spmd

    in_maps = make_in_maps(**inputs)
    nc = _get_nc()
    res = run_bass_kernel_spmd(nc, in_maps, list(range(N_CORES)), trace=trace)
    partials = [r["out"] for r in res.results]
    return assemble(partials, inputs["bo"], inputs["bv"], inputs["Wo"]), res


def kernel(**inputs):
    out, _ = run(inputs, trace=False)
    return out
